# revision 1
# baseline (speedup 1.0000x reference)
"""Trainium2 Bass kernel for nn_ClassificationAverageModel.

reference:
    pooled = mean(embeddings[x], axis=1)        # (B, D)
    logits = pooled @ W.T + b                   # (B, C)
    out    = softmax(logits, axis=1)

Strategy (memory-regime):
  softmax(mean_w(E[x]) @ W.T + b) == softmax(sum_w((E @ (W.T/L))[x]) + b)
so we first project the embedding table down to class space
(P = E @ W.T / L, shape V x C), then gather 80B projected rows instead of
1200B embedding rows -- ~13x less gather traffic.

Distribution across the 8 cores: vocab-sharded. Core c owns table rows
[c*V/8, (c+1)*V/8): it projects its shard (PE transpose + matmul), then
dma_gather's the in-shard words of ALL docs (int16 local indices fit the
shard), pools them into per-doc partial sums with selection-matrix matmuls
(0/1 matrices built on-device with is_equal against an iota), and a
ReduceScatter(add) hands every core the complete sums for its 1/8 of the
batch, where bias + softmax finish the job.

Host-side prep is only index bookkeeping: tokens are grouped per 128-doc
tile with a fixed per-tile budget (pad tokens point at an all-zero table
row), and laid out in dma_gather's 16-wrap index / 128-wrap output orders.
"""

import numpy as np

import concourse.bass as bass
import concourse.mybir as mybir
import concourse.tile as tile
from concourse import bacc, library_config
from concourse.bass_utils import run_bass_kernel_spmd
from concourse.masks import make_identity
from concourse.vector_clock import ScopedClock

F32 = mybir.dt.float32
I16 = mybir.dt.int16
I32 = mybir.dt.int32

NCORES = 8


class PatchedTileContext(tile.TileContext):
    """Split the kernel-tail drain's sem waits: walrus TRN2 CTRL codegen
    rejects drain instructions carrying more than ~2 sync waits."""

    def _drain_and_barrier(self, tick_clock, wait_clock):
        drain_inst = self.nc.sync.drain()
        wait_clock.add_sem_waits(
            drain_inst.ins, ScopedClock({None: tick_clock.global_clock})
        )
        si = drain_inst.ins.sync_info
        waits = list(si.on_wait) if si is not None else []
        if len(waits) > 1:
            si.on_wait = waits[:1]
            for w in waits[1:]:
                d2 = self.nc.sync.drain()
                si2 = d2.ins.sync_info
                if si2 is None:
                    d2.ins.sync_info = mybir.SyncInfo(on_wait=[w], on_update=[])
                else:
                    si2.on_wait = [w]
        self.nc.all_engine_barrier()
        popped = self.nc._tile_sem_poison_stack.pop()
        assert popped is self._sem_poison
        self.nc.clear_and_free_semaphores(list(self.sems.allocated().values()))
        self.nc.all_engine_barrier()


class Cfg:
    def __init__(self, vocab=100000, embed=300, ncls=20, batch=4096, doclen=200,
                 tile_budget=3584, dt_per_call=4):
        assert vocab % NCORES == 0 and batch % (128 * NCORES) == 0
        self.vocab, self.embed, self.ncls = vocab, embed, ncls
        self.batch, self.doclen = batch, doclen
        self.vsh = vocab // NCORES                  # shard rows per core
        self.pad_idx = self.vsh                     # all-zero row
        self.vsh_pad = -(-(self.vsh + 1) // 128) * 128
        self.prow = 64                              # padded P row elems (256B)
        self.ndt = batch // 128                     # doc tiles
        assert tile_budget % 128 == 0
        self.tile_budget = tile_budget              # tokens per doc tile
        self.cols_per_dt = tile_budget // 128
        self.dt_per_call = min(dt_per_call, self.ndt)
        assert self.ndt % self.dt_per_call == 0
        self.ncalls = self.ndt // self.dt_per_call
        self.call_tokens = tile_budget * self.dt_per_call
        self.call_cols = self.call_tokens // 128
        self.docs_out = batch // NCORES             # docs per core output
        self.kchunks = [(k * 128, min(128, embed - k * 128))
                        for k in range(-(-embed // 128))]

    def key(self):
        return (self.vocab, self.embed, self.ncls, self.batch, self.doclen,
                self.tile_budget, self.dt_per_call)


def _build_program(cfg: Cfg, repeats: int = 1, stages: str = "full", loop_iters: int = 0):
    c = cfg
    nc = bacc.Bacc("TRN2", target_bir_lowering=False, debug=False,
                   num_devices=NCORES, num_swdge_queues=4)
    e_sh = nc.dram_tensor("e_sh", [c.vsh, c.embed], F32, kind="ExternalInput")
    w_in = nc.dram_tensor("w_in", [c.ncls, c.embed], F32, kind="ExternalInput")
    b_in = nc.dram_tensor("b_in", [128, c.ncls], F32, kind="ExternalInput")
    gidx = nc.dram_tensor("gidx", [128, c.ndt * c.tile_budget // 16], I16,
                          kind="ExternalInput")
    dmod = nc.dram_tensor("dmod", [128, c.ndt * c.cols_per_dt], F32,
                          kind="ExternalInput")
    out = nc.dram_tensor("out", [c.docs_out, c.ncls], F32,
                         kind="ExternalOutput")
    p_d = nc.dram_tensor("p_d", [c.vsh_pad, c.prow], F32)

    nk = len(c.kchunks)
    with PatchedTileContext(nc) as tc:
        with (
            tc.tile_pool(name="const", bufs=1) as cpool,
            tc.tile_pool(name="dram", bufs=1, space="DRAM") as dram,
        ):
            nc.gpsimd.load_library(library_config.mlp)

            ident = cpool.tile([128, 128], F32)
            make_identity(nc, ident[:])

            iota_i = cpool.tile([128, 128], I32)
            nc.gpsimd.iota(iota_i[:], pattern=[[1, 128]], base=0,
                           channel_multiplier=0)
            iota_f = cpool.tile([128, 128], F32)
            nc.vector.tensor_copy(out=iota_f[:], in_=iota_i[:])

            b_t = cpool.tile([128, c.ncls], F32)
            nc.sync.dma_start(out=b_t[:], in_=b_in[:])

            # ---- W.T / doclen, laid out as K-chunks side by side ----
            w_sb = cpool.tile([128, c.embed], F32)
            nc.sync.dma_start(out=w_sb[:c.ncls, :], in_=w_in[:])
            wt_sb = cpool.tile([128, nk * c.ncls], F32)
            with tc.tile_pool(name="wps", bufs=nk, space="PSUM") as wps:
                for k, (k0, kw) in enumerate(c.kchunks):
                    wt_ps = wps.tile([128, 128], F32)
                    nc.tensor.transpose(
                        out=wt_ps[:kw, :c.ncls],
                        in_=w_sb[:c.ncls, k0:k0 + kw],
                        identity=ident[:c.ncls, :c.ncls],
                    )
                    nc.scalar.mul(
                        out=wt_sb[:kw, k * c.ncls:(k + 1) * c.ncls],
                        in_=wt_ps[:kw, :c.ncls],
                        mul=1.0 / c.doclen,
                    )

            # ---- zero the pad rows of P ----
            zpad = cpool.tile([128, c.prow], F32)
            nc.vector.memset(zpad[:], 0.0)
            npad = c.vsh_pad - c.vsh
            nc.sync.dma_start(out=p_d[c.vsh:c.vsh_pad, :], in_=zpad[:npad, :])

            # ---- body (repeatable for device-time measurement) ----
            import contextlib
            # NOTE: collectives inside For_i desync the mesh; only time
            # stages up to "pool" with loop_iters.
            assert not (loop_iters and stages == "full")
            _loop = tc.For_i(0, loop_iters, 1) if loop_iters else contextlib.nullcontext()
            with _loop:
                for _rep in range(repeats):
                  # ---- phase 1: P = E_shard @ (W.T/L), vocab-chunked ----
                  nchunks = -(-c.vsh // 128)
                  with (
                      tc.tile_pool(name="ep", bufs=3) as epool,
                      tc.tile_pool(name="lh", bufs=2) as lpool,
                      tc.tile_pool(name="po", bufs=3) as opool,
                      tc.tile_pool(name="tps", bufs=4, space="PSUM") as tpool,
                      tc.tile_pool(name="pps", bufs=2, space="PSUM") as ppool,
                  ):
                      for ch in range(nchunks):
                          r0 = ch * 128
                          rows = min(128, c.vsh - r0)
                          e_t = epool.tile([128, c.embed], F32)
                          nc.sync.dma_start(out=e_t[:rows, :], in_=e_sh[r0:r0 + rows, :])
                          lh = lpool.tile([128, nk * 128], F32)
                          for k, (k0, kw) in enumerate(c.kchunks):
                              tp = tpool.tile([128, 128], F32)
                              nc.tensor.transpose(
                                  out=tp[:kw, :rows],
                                  in_=e_t[:rows, k0:k0 + kw],
                                  identity=ident[:rows, :rows],
                              )
                              nc.vector.tensor_copy(
                                  out=lh[:kw, k * 128:k * 128 + rows],
                                  in_=tp[:kw, :rows],
                              )
                          pp = ppool.tile([128, c.ncls], F32)
                          for k, (k0, kw) in enumerate(c.kchunks):
                              nc.tensor.matmul(
                                  out=pp[:rows, :],
                                  lhsT=lh[:kw, k * 128:k * 128 + rows],
                                  rhs=wt_sb[:kw, k * c.ncls:(k + 1) * c.ncls],
                                  start=(k == 0),
                                  stop=(k == nk - 1),
                              )
                          po = opool.tile([128, c.ncls], F32)
                          nc.scalar.copy(out=po[:rows, :], in_=pp[:rows, :])
                          nc.sync.dma_start(out=p_d[r0:r0 + rows, :c.ncls],
                                            in_=po[:rows, :])

                  # ---- phase 2: gather + selection-matmul pooling ----
                  if stages == "proj":
                      continue
                  partials_d = dram.tile([c.batch, c.ncls], F32)
                  keep_d = None
                  if stages == "gather":
                      keep_d = dram.tile([128, c.ncalls], F32, tag="keep_d")
                  with (
                      tc.tile_pool(name="gi", bufs=2) as gipool,
                      tc.tile_pool(name="gb", bufs=2) as gbpool,
                      tc.tile_pool(name="dm", bufs=2) as dmpool,
                      tc.tile_pool(name="sel", bufs=3) as selpool,
                      tc.tile_pool(name="pt", bufs=3) as ptpool,
                      tc.tile_pool(name="dps", bufs=4, space="PSUM") as dpool,
                  ):
                      cols16 = c.call_tokens // 16
                      for call in range(c.ncalls):
                          gi_t = gipool.tile([128, cols16], I16)
                          nc.sync.dma_start(
                              out=gi_t[:],
                              in_=gidx[:, call * cols16:(call + 1) * cols16])
                          g_t = gbpool.tile([128, c.call_cols * c.prow], F32)
                          g3 = g_t[:].rearrange("p (s e) -> p s e", e=c.prow)
                          # 1024-idx sub-gathers with single_packet=True: 64
                          # descriptors per engine -- the HW packet cap. One
                          # packed packet per engine amortizes the per-packet
                          # ring context switch that dominates small-element
                          # gathers (single_packet=False was ~7x slower).
                          GSUB = 1024
                          for j in range(c.call_tokens // GSUB):
                              nc.gpsimd.dma_gather(
                                  out_ap=g3[:, j * (GSUB // 128):(j + 1) * (GSUB // 128), :],
                                  in_ap=p_d[:],
                                  idxs_ap=gi_t[:, j * (GSUB // 16):(j + 1) * (GSUB // 16)],
                                  num_idxs=GSUB,
                                  num_idxs_reg=GSUB,
                                  elem_size=c.prow,
                                  single_packet=True,
                                  queue_num=j % 4,
                              )
                          if stages == "gather":
                              nc.sync.dma_start(out=keep_d[:, call:call + 1],
                                                in_=g_t[:, 0:1])
                              continue
                          dm_t = dmpool.tile([128, c.call_cols], F32)
                          nc.sync.dma_start(
                              out=dm_t[:],
                              in_=dmod[:, call * c.call_cols:(call + 1) * c.call_cols])
                          for dtl in range(c.dt_per_call):
                              dt = call * c.dt_per_call + dtl
                              pdt = dpool.tile([128, c.ncls], F32)
                              for sl in range(c.cols_per_dt):
                                  s = dtl * c.cols_per_dt + sl
                                  sel = selpool.tile([128, 128], F32)
                                  nc.vector.tensor_tensor(
                                      out=sel[:],
                                      in0=dm_t[:, s:s + 1].to_broadcast([128, 128]),
                                      in1=iota_f[:],
                                      op=mybir.AluOpType.is_equal,
                                  )
                                  nc.tensor.matmul(
                                      out=pdt[:],
                                      lhsT=sel[:],
                                      rhs=g3[:, s, :c.ncls],
                                      start=(sl == 0),
                                      stop=(sl == c.cols_per_dt - 1),
                                  )
                              pt = ptpool.tile([128, c.ncls], F32)
                              nc.scalar.copy(out=pt[:], in_=pdt[:])
                              nc.sync.dma_start(
                                  out=partials_d[dt * 128:(dt + 1) * 128, :],
                                  in_=pt[:])

                  # ---- collective + bias + softmax ----
                  if stages in ("gather", "pool"):
                      continue
                  rs_d = dram.tile([c.docs_out, c.ncls], F32)
                  nc.gpsimd.collective_compute(
                      "ReduceScatter",
                      mybir.AluOpType.add,
                      replica_groups=[list(range(NCORES))],
                      ins=[partials_d.opt()],
                      outs=[rs_d.opt()],
                  )
                  with (
                      tc.tile_pool(name="sm", bufs=2) as smpool,
                      tc.tile_pool(name="sms", bufs=2) as sspool,
                  ):
                      for t in range(c.docs_out // 128):
                          lt = smpool.tile([128, c.ncls], F32)
                          nc.sync.dma_start(out=lt[:],
                                            in_=rs_d[t * 128:(t + 1) * 128, :])
                          nc.vector.tensor_tensor(out=lt[:], in0=lt[:], in1=b_t[:],
                                                  op=mybir.AluOpType.add)
                          nmx = sspool.tile([128, 1], F32)
                          nc.vector.tensor_reduce(out=nmx[:], in_=lt[:],
                                                  op=mybir.AluOpType.max,
                                                  axis=mybir.AxisListType.X,
                                                  negate=True)
                          ex = smpool.tile([128, c.ncls], F32)
                          nc.scalar.activation(out=ex[:], in_=lt[:],
                                               func=mybir.ActivationFunctionType.Exp,
                                               bias=nmx[:], scale=1.0)
                          sm = sspool.tile([128, 1], F32)
                          nc.vector.reduce_sum(out=sm[:], in_=ex[:],
                                               axis=mybir.AxisListType.X)
                          rc = sspool.tile([128, 1], F32)
                          nc.vector.reciprocal(out=rc[:], in_=sm[:])
                          ot = smpool.tile([128, c.ncls], F32)
                          nc.vector.tensor_scalar_mul(out=ot[:], in0=ex[:],
                                                      scalar1=rc[:])
                          nc.sync.dma_start(out=out[t * 128:(t + 1) * 128, :],
                                            in_=ot[:])
    nc.compile()
    return nc


def _prep_index_inputs(cfg: Cfg, x: np.ndarray):
    """Per-core gather indices (16-wrap int16) and doc-id-mod-128 planes
    (128-wrap f32). Returns (gidx[8], dmod[8], max_count)."""
    c = cfg
    flat_v = x.reshape(-1).astype(np.int64)
    tok_doc = np.repeat(np.arange(c.batch, dtype=np.int64), c.doclen)
    core_of = flat_v // c.vsh
    local = (flat_v - core_of * c.vsh).astype(np.int32)
    dt_of = tok_doc >> 7
    key = core_of * c.ndt + dt_of
    counts = np.bincount(key, minlength=NCORES * c.ndt)
    max_count = int(counts.max())
    if max_count > c.tile_budget:
        return None, None, max_count
    order = np.argsort(key, kind="stable")
    key_s = key[order]
    # position within each (core, dt) group
    group_start = np.zeros(NCORES * c.ndt, np.int64)
    np.cumsum(counts[:-1], out=group_start[1:])
    pos = np.arange(key.size, dtype=np.int64) - group_start[key_s]
    slot = (key_s % c.ndt) * c.tile_budget + pos      # slot within core
    core_s = key_s // c.ndt

    nslots = c.ndt * c.tile_budget
    gflat = np.full((NCORES, nslots), c.pad_idx, np.int32)
    dflat = np.full((NCORES, nslots), -1.0, np.float32)
    gflat[core_s, slot] = local[order]
    dflat[core_s, slot] = (tok_doc[order] & 127).astype(np.float32)

    # Sort tokens by table row within each doc-tile block: gather reads
    # become near-sequential (HBM row-buffer hits) instead of random.
    # Pooling is order-agnostic as long as the doc-id plane is permuted
    # identically and tokens stay within their doc-tile budget.
    gv = gflat.reshape(NCORES, c.ndt, c.tile_budget)
    dv = dflat.reshape(NCORES, c.ndt, c.tile_budget)
    perm = np.argsort(gv, axis=2, kind="stable")
    gflat = np.take_along_axis(gv, perm, axis=2).reshape(NCORES, nslots)
    dflat = np.take_along_axis(dv, perm, axis=2).reshape(NCORES, nslots)

    # 16-wrap per call: token j of a call -> [j%16, call*cols16 + j//16]
    g16 = (gflat.reshape(NCORES, c.ncalls, c.call_tokens // 16, 16)
           .transpose(0, 1, 3, 2))           # (8, ncalls, 16, cols16)
    g16 = np.concatenate([g16[:, i] for i in range(c.ncalls)], axis=2)
    gidx = np.tile(g16, (1, 8, 1)).astype(np.int16)   # (8, 128, totcols)

    # 128-wrap: token j of a call -> [j%128, call*call_cols + j//128]
    d128 = (dflat.reshape(NCORES, c.ncalls, c.call_cols, 128)
            .transpose(0, 1, 3, 2))          # (8, ncalls, 128, call_cols)
    dmod = np.concatenate([d128[:, i] for i in range(c.ncalls)], axis=2)
    dmod = np.ascontiguousarray(dmod, dtype=np.float32)  # (8, 128, cols)
    return gidx, dmod, max_count


_PROGRAM_CACHE: dict = {}


def _get_program(cfg: Cfg, repeats: int = 1, stages: str = "full", loop_iters: int = 0):
    k = (cfg.key(), repeats, stages, loop_iters)
    if k not in _PROGRAM_CACHE:
        _PROGRAM_CACHE[k] = _build_program(cfg, repeats, stages, loop_iters)
    return _PROGRAM_CACHE[k]


def run(embeddings, W, b, x, cfg: Cfg | None = None, trace=False, repeats: int = 1):
    if cfg is None:
        cfg = Cfg()
    embeddings = np.ascontiguousarray(np.asarray(embeddings, dtype=np.float32))
    W = np.ascontiguousarray(np.asarray(W, dtype=np.float32))
    b = np.asarray(b, dtype=np.float32).reshape(1, -1)
    x = np.asarray(x)

    gidx, dmod, max_count = _prep_index_inputs(cfg, x)
    while gidx is None:  # budget overflow (non-uniform input): grow and retry
        bigger = -(-max_count // 128) * 128
        cfg = Cfg(cfg.vocab, cfg.embed, cfg.ncls, cfg.batch, cfg.doclen,
                  tile_budget=bigger, dt_per_call=1)
        gidx, dmod, max_count = _prep_index_inputs(cfg, x)

    nc = _get_program(cfg, repeats)
    b_tiled = np.tile(b, (128, 1))
    in_maps = []
    for c in range(NCORES):
        in_maps.append({
            "e_sh": embeddings[c * cfg.vsh:(c + 1) * cfg.vsh],
            "w_in": W,
            "b_in": b_tiled,
            "gidx": gidx[c],
            "dmod": dmod[c],
        })
    res = run_bass_kernel_spmd(nc, in_maps, list(range(NCORES)),
                               trace=trace)
    out = np.concatenate([res.results[c]["out"] for c in range(NCORES)],
                         axis=0)
    return out, res


def kernel(embeddings, W, b, x):
    out, _ = run(embeddings, W, b, x)
    return out



# revision 3
# speedup vs baseline: 9.4131x; 9.4131x over previous
"""Trainium2 Bass kernel for nn_ClassificationAverageModel.

reference:
    pooled = mean(embeddings[x], axis=1)        # (B, D)
    logits = pooled @ W.T + b                   # (B, C)
    out    = softmax(logits, axis=1)

Strategy (memory-regime):
  softmax(mean_w(E[x]) @ W.T + b) == softmax(sum_w((E @ (W.T/L))[x]) + b)
so we first project the embedding table down to class space
(P = E @ W.T / L, shape V x C), then gather 80B projected rows instead of
1200B embedding rows -- ~13x less gather traffic.

Distribution across the 8 cores: vocab-sharded. Core c owns table rows
[c*V/8, (c+1)*V/8): it projects its shard (PE transpose + matmul), then
dma_gather's the in-shard words of ALL docs (int16 local indices fit the
shard), pools them into per-doc partial sums with selection-matrix matmuls
(0/1 matrices built on-device with is_equal against an iota), and a
ReduceScatter(add) hands every core the complete sums for its 1/8 of the
batch, where bias + softmax finish the job.

Host-side prep is only index bookkeeping: tokens are grouped per 128-doc
tile with a fixed per-tile budget (pad tokens point at an all-zero table
row), and laid out in dma_gather's 16-wrap index / 128-wrap output orders.
"""

import numpy as np

import concourse.bass as bass
import concourse.mybir as mybir
import concourse.tile as tile
from concourse import bacc, library_config
from concourse.bass_utils import run_bass_kernel_spmd
from concourse.masks import make_identity
from concourse.vector_clock import ScopedClock

F32 = mybir.dt.float32
I16 = mybir.dt.int16
I32 = mybir.dt.int32

NCORES = 8


class PatchedTileContext(tile.TileContext):
    """Split the kernel-tail drain's sem waits: walrus TRN2 CTRL codegen
    rejects drain instructions carrying more than ~2 sync waits."""

    def _drain_and_barrier(self, tick_clock, wait_clock):
        drain_inst = self.nc.sync.drain()
        wait_clock.add_sem_waits(
            drain_inst.ins, ScopedClock({None: tick_clock.global_clock})
        )
        si = drain_inst.ins.sync_info
        waits = list(si.on_wait) if si is not None else []
        if len(waits) > 1:
            si.on_wait = waits[:1]
            for w in waits[1:]:
                d2 = self.nc.sync.drain()
                si2 = d2.ins.sync_info
                if si2 is None:
                    d2.ins.sync_info = mybir.SyncInfo(on_wait=[w], on_update=[])
                else:
                    si2.on_wait = [w]
        self.nc.all_engine_barrier()
        popped = self.nc._tile_sem_poison_stack.pop()
        assert popped is self._sem_poison
        self.nc.clear_and_free_semaphores(list(self.sems.allocated().values()))
        self.nc.all_engine_barrier()


class Cfg:
    def __init__(self, vocab=100000, embed=300, ncls=20, batch=4096, doclen=200,
                 tile_budget=3584, dt_per_call=4):
        assert vocab % NCORES == 0 and batch % (128 * NCORES) == 0
        self.vocab, self.embed, self.ncls = vocab, embed, ncls
        self.batch, self.doclen = batch, doclen
        self.vsh = vocab // NCORES                  # shard rows per core
        self.pad_idx = self.vsh                     # all-zero row
        self.vsh_pad = -(-(self.vsh + 1) // 128) * 128
        self.prow = 64                              # padded P row elems (256B)
        self.ndt = batch // 128                     # doc tiles
        assert tile_budget % 128 == 0
        self.tile_budget = tile_budget              # tokens per doc tile
        self.cols_per_dt = tile_budget // 128
        self.dt_per_call = min(dt_per_call, self.ndt)
        assert self.ndt % self.dt_per_call == 0
        self.ncalls = self.ndt // self.dt_per_call
        self.call_tokens = tile_budget * self.dt_per_call
        self.call_cols = self.call_tokens // 128
        self.docs_out = batch // NCORES             # docs per core output
        self.kchunks = [(k * 128, min(128, embed - k * 128))
                        for k in range(-(-embed // 128))]

    def key(self):
        return (self.vocab, self.embed, self.ncls, self.batch, self.doclen,
                self.tile_budget, self.dt_per_call)


def _build_program(cfg: Cfg, repeats: int = 1, stages: str = "full", loop_iters: int = 0):
    c = cfg
    nc = bacc.Bacc("TRN2", target_bir_lowering=False, debug=False,
                   num_devices=NCORES, num_swdge_queues=4)
    e_sh = nc.dram_tensor("e_sh", [c.vsh, c.embed], F32, kind="ExternalInput")
    w_in = nc.dram_tensor("w_in", [c.ncls, c.embed], F32, kind="ExternalInput")
    b_in = nc.dram_tensor("b_in", [128, c.ncls], F32, kind="ExternalInput")
    gidx = nc.dram_tensor("gidx", [128, c.ndt * c.tile_budget // 16], I16,
                          kind="ExternalInput")
    dmod = nc.dram_tensor("dmod", [128, c.ndt * c.cols_per_dt], F32,
                          kind="ExternalInput")
    out = nc.dram_tensor("out", [c.docs_out, c.ncls], F32,
                         kind="ExternalOutput")
    p_d = nc.dram_tensor("p_d", [c.vsh_pad, c.prow], F32)

    nk = len(c.kchunks)
    with PatchedTileContext(nc) as tc:
        with (
            tc.tile_pool(name="const", bufs=1) as cpool,
            tc.tile_pool(name="dram", bufs=1, space="DRAM") as dram,
        ):
            nc.gpsimd.load_library(library_config.mlp)

            ident = cpool.tile([128, 128], F32)
            make_identity(nc, ident[:])

            iota_i = cpool.tile([128, 128], I32)
            nc.gpsimd.iota(iota_i[:], pattern=[[1, 128]], base=0,
                           channel_multiplier=0)
            iota_f = cpool.tile([128, 128], F32)
            nc.vector.tensor_copy(out=iota_f[:], in_=iota_i[:])

            b_t = cpool.tile([128, c.ncls], F32)
            nc.sync.dma_start(out=b_t[:], in_=b_in[:])

            # ---- W.T / doclen, laid out as K-chunks side by side ----
            w_sb = cpool.tile([128, c.embed], F32)
            nc.sync.dma_start(out=w_sb[:c.ncls, :], in_=w_in[:])
            wt_sb = cpool.tile([128, nk * c.ncls], F32)
            with tc.tile_pool(name="wps", bufs=nk, space="PSUM") as wps:
                for k, (k0, kw) in enumerate(c.kchunks):
                    wt_ps = wps.tile([128, 128], F32)
                    nc.tensor.transpose(
                        out=wt_ps[:kw, :c.ncls],
                        in_=w_sb[:c.ncls, k0:k0 + kw],
                        identity=ident[:c.ncls, :c.ncls],
                    )
                    nc.scalar.mul(
                        out=wt_sb[:kw, k * c.ncls:(k + 1) * c.ncls],
                        in_=wt_ps[:kw, :c.ncls],
                        mul=1.0 / c.doclen,
                    )

            # ---- zero the pad rows of P ----
            zpad = cpool.tile([128, c.prow], F32)
            nc.vector.memset(zpad[:], 0.0)
            npad = c.vsh_pad - c.vsh
            nc.sync.dma_start(out=p_d[c.vsh:c.vsh_pad, :], in_=zpad[:npad, :])

            # ---- body (repeatable for device-time measurement) ----
            import contextlib
            # NOTE: collectives inside For_i desync the mesh; only time
            # stages up to "pool" with loop_iters.
            assert not (loop_iters and stages == "full")
            _loop = tc.For_i(0, loop_iters, 1) if loop_iters else contextlib.nullcontext()
            with _loop:
                for _rep in range(repeats):
                  # ---- phase 1: P = E_shard @ (W.T/L), vocab-chunked ----
                  nchunks = -(-c.vsh // 128)
                  with (
                      tc.tile_pool(name="ep", bufs=3) as epool,
                      tc.tile_pool(name="lh", bufs=2) as lpool,
                      tc.tile_pool(name="po", bufs=3) as opool,
                      tc.tile_pool(name="tps", bufs=4, space="PSUM") as tpool,
                      tc.tile_pool(name="pps", bufs=2, space="PSUM") as ppool,
                  ):
                      for ch in range(nchunks):
                          r0 = ch * 128
                          rows = min(128, c.vsh - r0)
                          e_t = epool.tile([128, c.embed], F32)
                          nc.sync.dma_start(out=e_t[:rows, :], in_=e_sh[r0:r0 + rows, :])
                          lh = lpool.tile([128, nk * 128], F32)
                          for k, (k0, kw) in enumerate(c.kchunks):
                              tp = tpool.tile([128, 128], F32)
                              nc.tensor.transpose(
                                  out=tp[:kw, :rows],
                                  in_=e_t[:rows, k0:k0 + kw],
                                  identity=ident[:rows, :rows],
                              )
                              nc.vector.tensor_copy(
                                  out=lh[:kw, k * 128:k * 128 + rows],
                                  in_=tp[:kw, :rows],
                              )
                          pp = ppool.tile([128, c.ncls], F32)
                          for k, (k0, kw) in enumerate(c.kchunks):
                              nc.tensor.matmul(
                                  out=pp[:rows, :],
                                  lhsT=lh[:kw, k * 128:k * 128 + rows],
                                  rhs=wt_sb[:kw, k * c.ncls:(k + 1) * c.ncls],
                                  start=(k == 0),
                                  stop=(k == nk - 1),
                              )
                          po = opool.tile([128, c.ncls], F32)
                          nc.scalar.copy(out=po[:rows, :], in_=pp[:rows, :])
                          nc.sync.dma_start(out=p_d[r0:r0 + rows, :c.ncls],
                                            in_=po[:rows, :])

                  # ---- phase 2: gather + selection-matmul pooling ----
                  if stages == "proj":
                      continue
                  partials_d = dram.tile([c.batch, c.ncls], F32)
                  keep_d = None
                  if stages == "gather":
                      keep_d = dram.tile([128, c.ncalls], F32, tag="keep_d")
                  with (
                      tc.tile_pool(name="gi", bufs=2) as gipool,
                      tc.tile_pool(name="gb", bufs=2) as gbpool,
                      tc.tile_pool(name="dm", bufs=2) as dmpool,
                      tc.tile_pool(name="sel", bufs=3) as selpool,
                      tc.tile_pool(name="pt", bufs=3) as ptpool,
                      tc.tile_pool(name="dps", bufs=4, space="PSUM") as dpool,
                  ):
                      cols16 = c.call_tokens // 16
                      for call in range(c.ncalls):
                          gi_t = gipool.tile([128, cols16], I16)
                          nc.sync.dma_start(
                              out=gi_t[:],
                              in_=gidx[:, call * cols16:(call + 1) * cols16])
                          g_t = gbpool.tile([128, c.call_cols * c.prow], F32)
                          g3 = g_t[:].rearrange("p (s e) -> p s e", e=c.prow)
                          # 1024-idx sub-gathers with single_packet=True: 64
                          # descriptors per engine -- the HW packet cap. One
                          # packed packet per engine amortizes the per-packet
                          # ring context switch that dominates small-element
                          # gathers (single_packet=False was ~7x slower).
                          GSUB = 1024
                          for j in range(c.call_tokens // GSUB):
                              nc.gpsimd.dma_gather(
                                  out_ap=g3[:, j * (GSUB // 128):(j + 1) * (GSUB // 128), :],
                                  in_ap=p_d[:],
                                  idxs_ap=gi_t[:, j * (GSUB // 16):(j + 1) * (GSUB // 16)],
                                  num_idxs=GSUB,
                                  num_idxs_reg=GSUB,
                                  elem_size=c.prow,
                                  single_packet=True,
                                  queue_num=j % 4,
                              )
                          if stages == "gather":
                              nc.sync.dma_start(out=keep_d[:, call:call + 1],
                                                in_=g_t[:, 0:1])
                              continue
                          dm_t = dmpool.tile([128, c.call_cols], F32)
                          nc.sync.dma_start(
                              out=dm_t[:],
                              in_=dmod[:, call * c.call_cols:(call + 1) * c.call_cols])
                          for dtl in range(c.dt_per_call):
                              dt = call * c.dt_per_call + dtl
                              pdt = dpool.tile([128, c.ncls], F32)
                              for sl in range(c.cols_per_dt):
                                  s = dtl * c.cols_per_dt + sl
                                  sel = selpool.tile([128, 128], F32)
                                  nc.vector.tensor_tensor(
                                      out=sel[:],
                                      in0=dm_t[:, s:s + 1].to_broadcast([128, 128]),
                                      in1=iota_f[:],
                                      op=mybir.AluOpType.is_equal,
                                  )
                                  nc.tensor.matmul(
                                      out=pdt[:],
                                      lhsT=sel[:],
                                      rhs=g3[:, s, :c.ncls],
                                      start=(sl == 0),
                                      stop=(sl == c.cols_per_dt - 1),
                                  )
                              pt = ptpool.tile([128, c.ncls], F32)
                              nc.scalar.copy(out=pt[:], in_=pdt[:])
                              nc.sync.dma_start(
                                  out=partials_d[dt * 128:(dt + 1) * 128, :],
                                  in_=pt[:])

                  # ---- collective + bias + softmax ----
                  if stages in ("gather", "pool"):
                      continue
                  rs_d = dram.tile([c.docs_out, c.ncls], F32)
                  nc.gpsimd.collective_compute(
                      "ReduceScatter",
                      mybir.AluOpType.add,
                      replica_groups=[list(range(NCORES))],
                      ins=[partials_d.opt()],
                      outs=[rs_d.opt()],
                  )
                  with (
                      tc.tile_pool(name="sm", bufs=2) as smpool,
                      tc.tile_pool(name="sms", bufs=2) as sspool,
                  ):
                      for t in range(c.docs_out // 128):
                          lt = smpool.tile([128, c.ncls], F32)
                          nc.sync.dma_start(out=lt[:],
                                            in_=rs_d[t * 128:(t + 1) * 128, :])
                          nc.vector.tensor_tensor(out=lt[:], in0=lt[:], in1=b_t[:],
                                                  op=mybir.AluOpType.add)
                          nmx = sspool.tile([128, 1], F32)
                          nc.vector.tensor_reduce(out=nmx[:], in_=lt[:],
                                                  op=mybir.AluOpType.max,
                                                  axis=mybir.AxisListType.X,
                                                  negate=True)
                          ex = smpool.tile([128, c.ncls], F32)
                          nc.scalar.activation(out=ex[:], in_=lt[:],
                                               func=mybir.ActivationFunctionType.Exp,
                                               bias=nmx[:], scale=1.0)
                          sm = sspool.tile([128, 1], F32)
                          nc.vector.reduce_sum(out=sm[:], in_=ex[:],
                                               axis=mybir.AxisListType.X)
                          rc = sspool.tile([128, 1], F32)
                          nc.vector.reciprocal(out=rc[:], in_=sm[:])
                          ot = smpool.tile([128, c.ncls], F32)
                          nc.vector.tensor_scalar_mul(out=ot[:], in0=ex[:],
                                                      scalar1=rc[:])
                          nc.sync.dma_start(out=out[t * 128:(t + 1) * 128, :],
                                            in_=ot[:])
    nc.compile()
    return nc


def _prep_index_inputs(cfg: Cfg, x: np.ndarray):
    """Per-core gather indices (16-wrap int16) and doc-id-mod-128 planes
    (128-wrap f32). Returns (gidx[8], dmod[8], max_count)."""
    c = cfg
    flat_v = x.reshape(-1).astype(np.int64)
    tok_doc = np.repeat(np.arange(c.batch, dtype=np.int64), c.doclen)
    core_of = flat_v // c.vsh
    local = (flat_v - core_of * c.vsh).astype(np.int32)
    dt_of = tok_doc >> 7
    key = core_of * c.ndt + dt_of
    counts = np.bincount(key, minlength=NCORES * c.ndt)
    max_count = int(counts.max())
    if max_count > c.tile_budget:
        return None, None, max_count
    order = np.argsort(key, kind="stable")
    key_s = key[order]
    # position within each (core, dt) group
    group_start = np.zeros(NCORES * c.ndt, np.int64)
    np.cumsum(counts[:-1], out=group_start[1:])
    pos = np.arange(key.size, dtype=np.int64) - group_start[key_s]
    slot = (key_s % c.ndt) * c.tile_budget + pos      # slot within core
    core_s = key_s // c.ndt

    nslots = c.ndt * c.tile_budget
    gflat = np.full((NCORES, nslots), c.pad_idx, np.int32)
    dflat = np.full((NCORES, nslots), -1.0, np.float32)
    gflat[core_s, slot] = local[order]
    dflat[core_s, slot] = (tok_doc[order] & 127).astype(np.float32)

    # Sort tokens by table row within each doc-tile block: gather reads
    # become near-sequential (HBM row-buffer hits) instead of random.
    # Pooling is order-agnostic as long as the doc-id plane is permuted
    # identically and tokens stay within their doc-tile budget.
    gv = gflat.reshape(NCORES, c.ndt, c.tile_budget)
    dv = dflat.reshape(NCORES, c.ndt, c.tile_budget)
    perm = np.argsort(gv, axis=2, kind="stable")
    gflat = np.take_along_axis(gv, perm, axis=2).reshape(NCORES, nslots)
    dflat = np.take_along_axis(dv, perm, axis=2).reshape(NCORES, nslots)

    # 16-wrap per call: token j of a call -> [j%16, call*cols16 + j//16]
    g16 = (gflat.reshape(NCORES, c.ncalls, c.call_tokens // 16, 16)
           .transpose(0, 1, 3, 2))           # (8, ncalls, 16, cols16)
    g16 = np.concatenate([g16[:, i] for i in range(c.ncalls)], axis=2)
    gidx = np.tile(g16, (1, 8, 1)).astype(np.int16)   # (8, 128, totcols)

    # 128-wrap: token j of a call -> [j%128, call*call_cols + j//128]
    d128 = (dflat.reshape(NCORES, c.ncalls, c.call_cols, 128)
            .transpose(0, 1, 3, 2))          # (8, ncalls, 128, call_cols)
    dmod = np.concatenate([d128[:, i] for i in range(c.ncalls)], axis=2)
    dmod = np.ascontiguousarray(dmod, dtype=np.float32)  # (8, 128, cols)
    return gidx, dmod, max_count


_PROGRAM_CACHE: dict = {}


def _get_program(cfg: Cfg, repeats: int = 1, stages: str = "full", loop_iters: int = 0):
    k = (cfg.key(), repeats, stages, loop_iters)
    if k not in _PROGRAM_CACHE:
        _PROGRAM_CACHE[k] = _build_program(cfg, repeats, stages, loop_iters)
    return _PROGRAM_CACHE[k]


def run(embeddings, W, b, x, cfg: Cfg | None = None, trace=False, repeats: int = 1,
        tmpdir=None):
    if cfg is None:
        cfg = Cfg()
    embeddings = np.ascontiguousarray(np.asarray(embeddings, dtype=np.float32))
    W = np.ascontiguousarray(np.asarray(W, dtype=np.float32))
    b = np.asarray(b, dtype=np.float32).reshape(1, -1)
    x = np.asarray(x)

    gidx, dmod, max_count = _prep_index_inputs(cfg, x)
    while gidx is None:  # budget overflow (non-uniform input): grow and retry
        bigger = -(-max_count // 128) * 128
        cfg = Cfg(cfg.vocab, cfg.embed, cfg.ncls, cfg.batch, cfg.doclen,
                  tile_budget=bigger, dt_per_call=1)
        gidx, dmod, max_count = _prep_index_inputs(cfg, x)

    nc = _get_program(cfg, repeats)
    b_tiled = np.tile(b, (128, 1))
    in_maps = []
    for c in range(NCORES):
        in_maps.append({
            "e_sh": embeddings[c * cfg.vsh:(c + 1) * cfg.vsh],
            "w_in": W,
            "b_in": b_tiled,
            "gidx": gidx[c],
            "dmod": dmod[c],
        })
    res = run_bass_kernel_spmd(nc, in_maps, list(range(NCORES)),
                               trace=trace, tmpdir=tmpdir)
    out = np.concatenate([res.results[c]["out"] for c in range(NCORES)],
                         axis=0)
    return out, res


def kernel(embeddings, W, b, x):
    out, _ = run(embeddings, W, b, x)
    return out

